# revision 29
# baseline (speedup 1.0000x reference)
"""Multi-head attention (RoPE) Trainium2 Bass kernel.

Problem: B=2, S=2048, d_model=1024, 16 heads x head_dim 64, fp32.

The reference faithfully replicates a torch rank-5 reshape bug: the
attention output [1,H,B,S,D] is transposed to [1,H,B,S,D]->(0,2,1,3,4)
and flat-reshaped to [B,S,H*D] BEFORE the Wo projection. Net semantics:
  out[b2, s2, :] = flatten(O[b, h, s0:s0+16, :]) @ Wo + bo
  with h = b2*8 + s2//256, b = (s2//128)%2, s0 = (s2%128)*16,
so the projection is PER-HEAD (contraction mixes 16 seq x 64 dims of one
head) and every (b,h) yields an independent [128, 1024] output block.

Sharding (8 cores): batch (2) x head groups (4 groups of 4 heads).
Per core: QKV slices via bf16 matmuls in transposed layout (bf16 x/W
halves the xt+weight DMA and LDWEIGHTS traffic; PSUM accumulation stays
fp32 and deep xt prefetch (bufs=12) keeps the PE fed), RoPE fully in
bf16 (cos/sin tables, +-1 rot matrix, raw q/k — exact in bf16 for rot,
and 2-byte operands unlock the DVE 2x mode on the rope muls), per-head
attention with unnormalized softmax in bf16 operands (ones-column
appended to V gives the denominator), normalize into ot64
[64, 4head, S] kept f32r (the phase-D rhs is a stride-16 gather —
2-byte strided reads stall the PE there, measured), then per-head
scrambled projection against full f32 Wo. Host places the 32
independent blocks and adds bo.

Measured: 252-255us vs the 294us fp32r baseline (rel err 6.5e-3,
gate 2e-2). Phase C is exp-floor-bound: the Scalar/ACT engine is ~95%
busy (1 elem/cycle/lane, 16.8M exps/core is irreducible). The drain
reshapes the softmax denominator [1,512]->[4,128] via DMA so the DVE
reciprocal runs 4 lanes wide (C-phase DVE 74us -> 36us), and both u
PSUM copies are emitted before the reciprocal chain so the banks free
before the next q tile's AV matmuls. Measured regressions kept OUT:
bf16 ot64/Wo in phase D (+30us, strided 2-byte rhs), phase-D split per
head pair (+17us, doubled Wo streaming), xt prefetch beyond 12 (flat),
and any restructuring that puts dependent chains into the in-order PE
stream (Q-proj/D-chunk/normalize interleaves all lost to p-state
collapse and PE-stream blocking).
"""

import numpy as np

import concourse.bass as bass
import concourse.tile as tile
from concourse import bacc, mybir
from concourse import bass_utils

F32 = mybir.dt.float32
MM_DT = mybir.dt.float32r  # matmul operand dtype (float32r: 1 cyc/row)
BF = mybir.dt.bfloat16  # QKV projection operands: halves xt/w DMA + ldweights

B, S, DM, H, HD = 2, 2048, 1024, 16, 64
N_CORES = 8
HG = 4          # head groups (tensor-parallel factor)
GD = DM // HG   # qkv dims per core = 256
NKC = DM // 128   # d_model contraction chunks = 8
NST = S // 512    # seq tiles of 512 = 4
NSK = S // 128    # seq_k chunks of 128 = 16
NQP = S // 1024   # seq_q pairs of 1024 = 2


def _emit(nc, tc, ap, debug=False):
    import contextlib

    ctx = contextlib.ExitStack()
    with ctx:
        consts = ctx.enter_context(tc.tile_pool(name="consts", bufs=1))
        big = ctx.enter_context(tc.tile_pool(name="big", bufs=1))

        # ---- constants / weights to SBUF, in first-use order: the first
        # Q-pass matmul only needs wq + one xt chunk, so the PE starts ~2us
        # in instead of waiting for the whole const burst. wv is needed at
        # the V pass (~+5us), rot at the first deferred rope (~+7us),
        # cos/sin at the first rope muls (~+8us), Wo only in phase D.
        # wq per-kc on the sync queue (the kc=0 chunk lands ~1us after the
        # prologue so the PE starts immediately); wk/wv per-kc on the gpsimd
        # queue (36ns dispatch, transfers run parallel to wq's) so neither
        # the k nor the v matmuls of st=0 stall.
        wq = consts.tile([128, NKC, GD], BF)
        wk = consts.tile([128, NKC, GD], BF)
        wv = consts.tile([128, NKC, GD], BF)
        for kc in range(NKC):
            nc.sync.dma_start(
                wq[:, kc, :],
                ap["wq"].rearrange("(kc p) m -> p kc m", p=128)[:, kc, :],
            )
            nc.gpsimd.dma_start(
                wk[:, kc, :],
                ap["wk"].rearrange("(kc p) m -> p kc m", p=128)[:, kc, :],
            )
            nc.gpsimd.dma_start(
                wv[:, kc, :],
                ap["wv"].rearrange("(kc p) m -> p kc m", p=128)[:, kc, :],
            )
        rot = consts.tile([128, 128], BF)
        nc.sync.dma_start(rot, ap["rot"])
        cosb = consts.tile([128, S], BF)
        nc.sync.dma_start(cosb, ap["cosb"])
        sinb = consts.tile([128, S], BF)
        nc.sync.dma_start(sinb, ap["sinb"])
        wo_sb = consts.tile([128, NKC, DM], MM_DT)
        nc.sync.dma_start(
            wo_sb, ap["wo"].rearrange("(cc p) m -> p cc m", p=128).bitcast(MM_DT)
        )
        bqc = consts.tile([128, 2], F32)
        nc.gpsimd.dma_start(bqc, ap["bq2"].rearrange("c p -> p c"))
        bkc = consts.tile([128, 2], F32)
        nc.gpsimd.dma_start(bkc, ap["bk2"].rearrange("c p -> p c"))
        bvb = consts.tile([128, GD], F32)
        nc.gpsimd.dma_start(bvb, ap["bv"].partition_broadcast(128))

        # ---- persistent activation buffers ----
        qe = [big.tile([128, S], BF, name=f"qe{mc}", tag=f"qe{mc}") for mc in range(2)]
        ke = [big.tile([128, S], BF, name=f"ke{mc}", tag=f"ke{mc}") for mc in range(2)]
        # V natural layout + ones column: [128 seq, kc, head, 65]
        vsb = big.tile([128, NSK, 4, 65], BF, name="vsb", tag="vsb")
        nc.vector.memset(vsb[:, :, :, 64:65], 1.0)
        # normalized attention output pre-scrambled for the phase-D dense
        # contraction: xb[p, cc, h, s2r] = O[h, s2r*16 + cc*2 + p//64, p%64],
        # i.e. contraction chunk cc holds Wo rows cc*128..cc*128+127 with
        # j = cc*2 + jo on partitions jo*64..jo*64+63.
        xb = big.tile([128, NKC, 4, S // 16], MM_DT, name="xb", tag="xb")

        # ================= Phase B: QKV projections + RoPE =================
        # Per seq tile: Q pass, K pass, V pass (xt tiles reused across all
        # three), with the QK->raw extraction emitted between K and V so the
        # DVE overlaps the V matmuls, and the rot+rope chain for tile st
        # deferred into tile st+1's Q pass so the PE never stalls on it.
        with (
            nc.named_scope("phaseB"),
            tc.tile_pool(name="xt", bufs=12) as xt_pool,
            tc.tile_pool(name="raw", bufs=3) as raw_pool,
            tc.tile_pool(name="t1", bufs=3) as t1_pool,
            tc.tile_pool(name="ps_qk", bufs=1, space="PSUM") as ps_qk,
            tc.tile_pool(name="ps_v", bufs=1, space="PSUM") as ps_v,
        ):
            for st in range(NST):
                sl = slice(st * 512, (st + 1) * 512)
                pqk = {}
                pv = {}
                for tgt in range(2):
                    for mc in range(2):
                        pqk[tgt, mc] = ps_qk.tile(
                            [128, 512], F32, name=f"pqk{tgt}{mc}", tag=f"qk{tgt}{mc}"
                        )
                for ss in range(4):
                    pv[ss] = ps_v.tile([128, GD], F32, name=f"pv{ss}", tag=f"v{ss}")
                for kc in range(NKC):
                    xt_kc = xt_pool.tile([128, 512], BF)
                    nc.scalar.dma_start(
                        xt_kc,
                        ap["xt"][kc * 128:(kc + 1) * 128, sl],
                    )
                    for tgt in range(2):
                        w_sb = wq if tgt == 0 else wk
                        for mc in range(2):
                            nc.tensor.matmul(
                                pqk[tgt, mc],
                                lhsT=w_sb[:, kc, mc * 128:(mc + 1) * 128],
                                rhs=xt_kc,
                                start=(kc == 0),
                                stop=(kc == NKC - 1),
                            )
                    for ss in range(4):
                        nc.tensor.matmul(
                            pv[ss],
                            lhsT=xt_kc[:, ss * 128:(ss + 1) * 128],
                            rhs=wv[:, kc, :],
                            start=(kc == 0),
                            stop=(kc == NKC - 1),
                        )
                # issue all accumulator drains first: the QK psum slots gate
                # the next seq tile's matmuls, so don't interleave the slower
                # RoPE chain between them
                raws = {}
                for tgt in range(2):
                    bias = bqc if tgt == 0 else bkc
                    for mc in range(2):
                        raw = raw_pool.tile([128, 512], BF, name=f"raw{tgt}{mc}", tag=f"raw{tgt}{mc}")
                        nc.vector.tensor_scalar_add(raw, pqk[tgt, mc], bias[:, mc:mc + 1])
                        raws[tgt, mc] = raw
                # rot reuses the pqk PSUM tags (freed by the raw extraction,
                # ~2.7us earlier than the pv tags) so the PE doesn't stall on
                # the bias-add drains; bias adds are emitted AFTER the rope
                # muls so the next tile's QK matmuls (gated on the rps death
                # = t1 muls) unblock as early as possible.
                rpss = {}
                for tgt in (1, 0):
                    for mc in range(2):
                        rps = ps_qk.tile(
                            [128, 512], F32, name=f"rps{tgt}{mc}", tag=f"qk{tgt}{mc}"
                        )
                        nc.tensor.matmul(rps, lhsT=rot, rhs=raws[tgt, mc], start=True, stop=True)
                        rpss[tgt, mc] = rps
                # k (tgt=1) first: phase C's first score matmuls need ke
                for tgt in (1, 0):
                    dst = qe if tgt == 0 else ke
                    for mc in range(2):
                        t1 = t1_pool.tile([128, 512], BF)
                        nc.vector.tensor_mul(t1, rpss[tgt, mc], sinb[:, sl])
                        d = dst[mc][:, sl]
                        nc.vector.tensor_mul(d, raws[tgt, mc], cosb[:, sl])
                        nc.vector.tensor_add(d, d, t1)
                for ss in range(4):
                    nc.vector.tensor_add(
                        vsb[:, st * 4 + ss, :, 0:64],
                        pv[ss].rearrange("p (h d) -> p h d", h=4),
                        bvb.rearrange("p (h d) -> p h d", h=4),
                    )

        if debug:
            for mc in range(2):
                nc.sync.dma_start(ap["qe_dbg"][mc], qe[mc].bitcast(F32))
                nc.sync.dma_start(ap["ke_dbg"][mc], ke[mc].bitcast(F32))
            nc.sync.dma_start(ap["v_dbg"], vsb.bitcast(F32))

        # ================= Phase C: attention =================
        # Scores are emitted in half-blocks hb = kc*2 + hi of 512 q each,
        # grouped 3 to a 3-bank PSUM tile so the exp ACTIVATE runs 1536 wide
        # (amortizes the ~220ns/inst ACT overhead; ACT is the phase C floor).
        GLAG = 2   # AV matmuls trail the exp groups by GLAG groups
        NHB = NSK * 2          # 32 half-blocks per (hc, qt)
        NG = (NHB + 2) // 3    # 11 groups: 10x3 + 1x2
        with (
            nc.named_scope("phaseC"),
            tc.tile_pool(name="e", bufs=GLAG + 3) as e_pool,
            tc.tile_pool(name="rcp", bufs=4) as rcp_pool,
            tc.tile_pool(name="usb", bufs=2) as usb_pool,
            tc.tile_pool(name="stg", bufs=2) as stg_pool,
            tc.tile_pool(name="rdram", bufs=4, space="DRAM") as rdram_pool,
            tc.tile_pool(name="ps_s", bufs=2, space="PSUM") as ps_s,
            tc.tile_pool(name="ps_u", bufs=1, space="PSUM") as ps_u,
        ):
            class _TS:
                pass

            def _emit_scores(ts, g):
                hbs = list(range(3 * g, min(3 * g + 3, NHB)))
                w = 512 * len(hbs)
                gp = ps_s.tile([128, 1536], F32, tag="sg", name="sg")
                for i, hb in enumerate(hbs):
                    kc, hi = divmod(hb, 2)
                    hpart = slice(hi * 64, (hi + 1) * 64)
                    nc.tensor.matmul(
                        gp[:, i * 512:(i + 1) * 512],
                        lhsT=ke[ts.hc][hpart, kc * 128:(kc + 1) * 128],
                        rhs=qe[ts.hc][hpart, ts.qsl],
                        start=True,
                        stop=True,
                    )
                e = e_pool.tile([128, 1536], BF, name="e", tag="e")
                nc.scalar.activation(
                    e[:, :w], gp[:, :w],
                    mybir.ActivationFunctionType.Exp, scale=0.125,
                )
                ts.egs[g] = e

            def _emit_av(ts, ge):
                # AV for every ka whose both half-blocks are exp'd in <= ge
                ka_avail = min((3 * (ge + 1)) // 2, NSK)
                for ka in range(ts.ka_done, ka_avail):
                    for hi in range(2):
                        hb = 2 * ka + hi
                        nc.tensor.matmul(
                            ts.u[hi],
                            lhsT=vsb[:, ka, ts.hc * 2 + hi, :],
                            rhs=ts.egs[hb // 3][:, (hb % 3) * 512:(hb % 3) * 512 + 512],
                            start=(ka == 0),
                            stop=(ka == NSK - 1),
                        )
                ts.ka_done = ka_avail
                for gd in list(ts.egs):
                    if min(3 * gd + 3, NHB) - 1 < 2 * ts.ka_done:
                        del ts.egs[gd]

            def _emit_drain(ts):
                # The two hi chains run on disjoint engine/queue pairs
                # (hi=0: DVE + sync queue, hi=1: gpsimd engine + queue) so
                # they overlap; the denominator row is DMA'd straight out of
                # PSUM so the reciprocal chain starts at AV-stop, parallel
                # with the u copies.
                # hi=1 first everywhere: its chain (gpsimd muls are slower)
                # is the critical path, so give it the earliest start
                usbs = {}
                for hi in (1, 0):
                    usb = usb_pool.tile([65, 512], F32, name="usb", tag=f"usb{hi}")
                    nc.vector.tensor_copy(usb, ts.u[hi])
                    usbs[hi] = usb
                dens = {}
                for hi in (1, 0):
                    # [1,512] -> [8,64]: reciprocal runs 8 lanes wide
                    den8 = rcp_pool.tile([8, 64], F32, tag=f"den8_{hi}", name="den8")
                    dq = nc.sync if hi == 0 else nc.gpsimd
                    dq.dma_start(den8, usbs[hi][64:65, :])
                    dens[hi] = den8
                for hi in (1, 0):
                    usb = usbs[hi]
                    dq = nc.sync if hi == 0 else nc.gpsimd
                    eng = nc.vector
                    r8 = rcp_pool.tile([8, 64], F32, tag=f"r8_{hi}", name="r8")
                    nc.vector.reciprocal(r8, dens[hi])
                    rd = rdram_pool.tile([8, 64], F32)
                    dq.dma_start(rd, r8)
                    dbc_sb = rcp_pool.tile([64, 512], F32, tag=f"dbc_sb{hi}", name="dbc_sb")
                    dq.dma_start(
                        dbc_sb,
                        rd.rearrange("(i s) q -> i (s q)", s=8)[0:1, :]
                        .partition_broadcast(64),
                    )
                    # normalize straight into the scrambled xb layout:
                    # q_local = s2r*16 + cc*2 + jo.  jo=0 lands on partitions
                    # 0-63 directly; jo=1 goes via a staging tile then a
                    # partition-shifting DMA to 64-127.
                    h = ts.hc * 2 + hi
                    s2rsl = slice(ts.qt * 32, (ts.qt + 1) * 32)
                    u_r = usb[0:64, :].rearrange(
                        "p (s2r cc two) -> p cc s2r two", s2r=32, cc=8, two=2
                    )
                    d_r = dbc_sb.rearrange(
                        "p (s2r cc two) -> p cc s2r two", s2r=32, cc=8, two=2
                    )
                    eng.tensor_mul(
                        xb[0:64, :, h, s2rsl], u_r[:, :, :, 0], d_r[:, :, :, 0]
                    )
                    stg = stg_pool.tile(
                        [64, NKC, 32], MM_DT, name="stg", tag=f"stg{hi}"
                    )
                    eng.tensor_mul(stg, u_r[:, :, :, 1], d_r[:, :, :, 1])
                    dq2 = nc.sync if hi == 0 else nc.gpsimd
                    dq2.dma_start(xb[64:128, :, h, s2rsl], stg)

            # Software-pipelined over the 8 (hc, qt) tiles: the previous
            # tile's trailing AV chunks + drain are emitted BETWEEN the
            # current tile's first score groups, so the ACT stream never
            # waits behind trailing AV matmuls on the in-order PE queue.
            prev = None
            for hc in range(2):
                for qt in range(NST):
                    ts = _TS()
                    ts.hc, ts.qt = hc, qt
                    ts.qsl = slice(qt * 512, (qt + 1) * 512)
                    ts.u = [
                        ps_u.tile([65, 512], F32, name=f"u{i}", tag=f"u{i}")
                        for i in range(2)
                    ]
                    ts.egs = {}
                    ts.ka_done = 0
                    for g in range(NG):
                        _emit_scores(ts, g)
                        if g < GLAG:
                            if prev is not None:
                                _emit_av(prev, NG - GLAG + g)
                                if g == GLAG - 1:
                                    _emit_drain(prev)
                                    prev = None
                        else:
                            _emit_av(ts, g - GLAG)
                    prev = ts
            _emit_av(prev, NG - 1)
            _emit_drain(prev)

        if debug:
            nc.sync.dma_start(ap["ot_dbg"], xb.bitcast(F32))

        # ====== Phase D: dense output projection against prestaged Wo ======
        # ypt[mc*128+p_out, h*128+s2r] = sum_cc sum_p Wo[cc*128+p, mc*128+p_out]
        #   * xb[p, cc, h, s2r] — full 128-row contraction, 8-deep accumulate.
        with (
            nc.named_scope("phaseD"),
            tc.tile_pool(name="ysb", bufs=3) as y_pool,
            tc.tile_pool(name="ps_y", bufs=4, space="PSUM") as ps_y,
        ):
            for mc in range(NKC):
                py = ps_y.tile([128, 512], F32, name="py", tag="py")
                for cc in range(NKC):
                    nc.tensor.matmul(
                        py,
                        lhsT=wo_sb[:, cc, mc * 128:(mc + 1) * 128],
                        rhs=xb[:, cc, :, :],
                        start=(cc == 0),
                        stop=(cc == NKC - 1),
                    )
                ysb = y_pool.tile([128, 512], F32, name="ysb", tag="ysb")
                nc.vector.tensor_copy(ysb, py)
                nc.sync.dma_start(ap["ypt"][mc * 128:(mc + 1) * 128, :], ysb)


def _build(debug=False):
    nc = bacc.Bacc("TRN2", target_bir_lowering=False, debug=False, num_devices=N_CORES)
    ap = {}
    ap["xt"] = nc.dram_tensor("xt", [DM, S], BF, kind="ExternalInput").ap()
    ap["wq"] = nc.dram_tensor("wq", [DM, GD], BF, kind="ExternalInput").ap()
    ap["wk"] = nc.dram_tensor("wk", [DM, GD], BF, kind="ExternalInput").ap()
    ap["wv"] = nc.dram_tensor("wv", [DM, GD], BF, kind="ExternalInput").ap()
    ap["wo"] = nc.dram_tensor("wo", [DM, DM], F32, kind="ExternalInput").ap()
    ap["bq2"] = nc.dram_tensor("bq2", [2, 128], F32, kind="ExternalInput").ap()
    ap["bk2"] = nc.dram_tensor("bk2", [2, 128], F32, kind="ExternalInput").ap()
    ap["bv"] = nc.dram_tensor("bv", [GD], F32, kind="ExternalInput").ap()
    ap["cosb"] = nc.dram_tensor("cosb", [128, S], BF, kind="ExternalInput").ap()
    ap["sinb"] = nc.dram_tensor("sinb", [128, S], BF, kind="ExternalInput").ap()
    ap["rot"] = nc.dram_tensor("rot", [128, 128], BF, kind="ExternalInput").ap()
    # per-core output: Y^T [1024, 512] (columns = 4 heads x 128 block rows)
    ap["ypt"] = nc.dram_tensor("ypt", [DM, 512], F32, kind="ExternalOutput").ap()
    if debug:
        ap["qe_dbg"] = nc.dram_tensor("qe_dbg", [2, 128, S], F32, kind="ExternalOutput").ap()
        ap["ke_dbg"] = nc.dram_tensor("ke_dbg", [2, 128, S], F32, kind="ExternalOutput").ap()
        ap["v_dbg"] = nc.dram_tensor("v_dbg", [128, NSK, 4, 65], F32, kind="ExternalOutput").ap()
        ap["ot_dbg"] = nc.dram_tensor("ot_dbg", [128, 8, 4, S // 16], F32, kind="ExternalOutput").ap()

    with tile.TileContext(nc) as tc:
        _emit(nc, tc, ap, debug=debug)
    nc.compile()
    return nc


_CACHE = {}


def _rope_tables():
    inv_freq = (1.0 / (10000.0 ** (np.arange(0, HD, 2, dtype=np.float32) / HD))).astype(np.float32)
    t = np.arange(S, dtype=np.float32)
    freqs = np.outer(t, inv_freq).astype(np.float32)  # [S, 32]
    emb = np.concatenate([freqs, freqs], axis=-1)  # [S, 64]
    cosT = np.cos(emb).astype(np.float32).T  # [64, S]
    sinT = np.sin(emb).astype(np.float32).T
    cosb = np.ascontiguousarray(np.concatenate([cosT, cosT], axis=0))  # [128, S]
    sinb = np.ascontiguousarray(np.concatenate([sinT, sinT], axis=0))
    return cosb, sinb


def _rot_matrix():
    p64 = np.zeros((HD, HD), dtype=np.float32)
    for i in range(32):
        p64[i, i + 32] = -1.0
        p64[i + 32, i] = 1.0
    p = np.zeros((128, 128), dtype=np.float32)
    p[0:64, 0:64] = p64
    p[64:128, 64:128] = p64
    return np.ascontiguousarray(p.T)  # lhsT = P^T


def kernel(x, Wq, bq, Wk, bk, Wv, bv, Wo, bo):
    x = np.asarray(x, dtype=np.float32)
    Wq, bq = np.asarray(Wq, np.float32), np.asarray(bq, np.float32)
    Wk, bk = np.asarray(Wk, np.float32), np.asarray(bk, np.float32)
    Wv, bv = np.asarray(Wv, np.float32), np.asarray(bv, np.float32)
    Wo, bo = np.asarray(Wo, np.float32), np.asarray(bo, np.float32)

    if "nc" not in _CACHE:
        _CACHE["nc"] = _build()
    nc = _CACHE["nc"]

    cosb, sinb = _rope_tables()
    rot = _rot_matrix()
    import ml_dtypes

    bf16 = ml_dtypes.bfloat16
    xt_b = [np.ascontiguousarray(x[b].T).astype(bf16) for b in range(B)]  # [DM, S]
    wo_c = np.ascontiguousarray(Wo)

    in_maps = []
    for c in range(N_CORES):
        b, hg = divmod(c, HG)
        sl = slice(hg * GD, (hg + 1) * GD)
        in_maps.append(
            {
                "xt": xt_b[b],
                "wq": np.ascontiguousarray(Wq[:, sl]).astype(bf16),
                "wk": np.ascontiguousarray(Wk[:, sl]).astype(bf16),
                "wv": np.ascontiguousarray(Wv[:, sl]).astype(bf16),
                "wo": wo_c,
                "bq2": np.ascontiguousarray(bq[sl].reshape(2, 128)),
                "bk2": np.ascontiguousarray(bk[sl].reshape(2, 128)),
                "bv": np.ascontiguousarray(bv[sl]),
                "cosb": cosb.astype(bf16),
                "sinb": sinb.astype(bf16),
                "rot": rot.astype(bf16),
            }
        )

    res = bass_utils.run_bass_kernel_spmd(nc, in_maps, core_ids=list(range(N_CORES)))
    _CACHE["last_results"] = res

    # Block placement: core (b, hg), local head hl -> global head h = hg*4+hl,
    # lands at out[h//8, (h%8)*256 + b*128 : +128, :].
    out = np.empty((B, S, DM), dtype=np.float32)
    for c in range(N_CORES):
        b, hg = divmod(c, HG)
        ypt = res.results[c]["ypt"]  # [1024, 512]
        for hl in range(4):
            h = hg * 4 + hl
            b2 = h // 8
            s2 = (h % 8) * 256 + b * 128
            out[b2, s2:s2 + 128, :] = ypt[:, hl * 128:(hl + 1) * 128].T
    out += bo[None, None, :]
    return out

